# revision 7
# baseline (speedup 1.0000x reference)
"""GCNConv on 8 TRN2 NeuronCores.

out = rownorm(A + I) @ (x @ W) + b   with A = dense scatter (set semantics)
    = [per dst row r: (sum_{c in dedup(nbr(r))} x[c] + x[r]) / (deg(r)+1)] @ W + b

Strategy (1D node partition, per the sharding hint):
  - host: dedup edges, partition dst rows into 8 contiguous blocks of 2048,
    degree-sort rows inside each core block into 16 tiles of 128 rows,
    build a padded-CSR gather-index array [128, sum(K_t)] per core
    (pad slots point at a zeroed row), plus 1/(deg+1) per row.
  - device (identical program on all 8 cores, different data):
      * cast x f32 -> fp16 into a DRAM scratch (one SWDGE cast DMA)
      * per 2-tile group: one indirect-DMA gather of neighbor rows into
        SBUF [128, K*32] fp16 (one descriptor per edge slot)
      * DVE halving-tree segment sum -> S [128,32] f32
      * PE transpose -> S^T, PE matmul S@W, scalar scale by 1/(deg+1),
        DVE bias add, DMA out
  - host: inverse-permute the 8x2048 row blocks into the full output.
"""

import numpy as np
from contextlib import ExitStack

N = 16384
E = 524288
D = 32
P = 128
NCORES = 8
RPC = N // NCORES          # rows per core = 2048
NTILES = RPC // P          # 16 tiles of 128 rows per core
GROUP = 2                  # tiles per gather instruction
ZROW = N                   # index of the zeroed pad row in the fp16 scratch

_CACHE = {}
_PREP_CACHE = {}
LAST_RESULTS = None        # BassKernelResults of the last run (for test.py)
_TRACE = False             # test.py can flip this for a profiled run


def _preprocess(edge_index):
    """Dedup edges, build per-core degree-sorted padded-CSR gather schedule."""
    ei = np.asarray(edge_index)
    key = ei.tobytes()
    if key in _PREP_CACHE:
        return _PREP_CACHE[key]

    dst = ei[0].astype(np.int64)
    src = ei[1].astype(np.int64)
    keys = np.unique(dst * N + src)          # set semantics
    d = (keys // N).astype(np.int64)
    s = (keys % N).astype(np.int32)
    rowptr = np.searchsorted(d, np.arange(N + 1)).astype(np.int64)
    deg = np.diff(rowptr)                    # distinct out-neighbors per row
    slots = (deg + 1).astype(np.int64)       # + self loop
    inv = (1.0 / slots).astype(np.float32)

    # per-core degree-descending row order
    perms = []
    for c in range(NCORES):
        rows = np.arange(c * RPC, (c + 1) * RPC)
        order = np.argsort(-slots[rows], kind="stable")
        perms.append(rows[order])

    # shared (SPMD) per-tile pad width: max slots across cores in that tile
    Ks = []
    for t in range(NTILES):
        m = max(int(slots[perms[c][t * P]]) for c in range(NCORES))
        Ks.append(max(m, 2))
    Ks = tuple(Ks)
    offs = np.concatenate([[0], np.cumsum(Ks)]).astype(np.int64)
    SUMK = int(offs[-1])

    ngroups = NTILES // GROUP
    idx_arrs, inv_arrs = [], []
    for c in range(NCORES):
        plain = np.full((P, SUMK), ZROW, np.int16)
        invt = np.zeros((P, NTILES), np.float32)
        pc = perms[c]
        for t in range(NTILES):
            o = int(offs[t])
            for p in range(P):
                r = int(pc[t * P + p])
                a, b = rowptr[r], rowptr[r + 1]
                k = int(b - a)
                plain[p, o:o + k] = s[a:b]
                plain[p, o + k] = r          # self loop slot
                invt[p, t] = inv[r]
        # dma_gather index format: per gather group, gathered position
        # i = j*128 + p reads wrapped[i%16, i//16]; wrapped block for group g
        # occupies columns [8*off_g, 8*(off_g+Kg)); replicated to all 128
        # partitions (one copy per GPSIMD core's partition group).
        idxw = np.empty((16, 8 * SUMK), np.int16)
        for g in range(ngroups):
            og = int(offs[g * GROUP])
            Kg = int(offs[(g + 1) * GROUP] - offs[g * GROUP])
            block = plain[:, og:og + Kg]          # [128, Kg]
            flat = block.T.reshape(-1)            # flat[j*128+p] = block[p, j]
            idxw[:, 8 * og:8 * (og + Kg)] = flat.reshape(-1, 16).T
        idx_arrs.append(np.ascontiguousarray(np.tile(idxw, (8, 1))))
        inv_arrs.append(invt)

    prep = {
        "Ks": Ks,
        "offs": offs,
        "SUMK": SUMK,
        "idx": idx_arrs,
        "inv": inv_arrs,
        "perm": perms,
    }
    _PREP_CACHE[key] = prep
    return prep


def _emit_dma_gather(nc, out_ap, in_ap, idxs_ap, num_idxs, elem_size, elem_step):
    """bass.dma_gather minus its elem_size_bytes%256 assert (that restriction
    is transpose-only; the real ISA constraint is the source stride, which is
    encoded in 256B units and satisfied by the 256B-pitch scratch)."""
    from concourse import mybir
    from concourse._compat import exact_div

    eng = nc.gpsimd
    assert in_ap.ap[0][0] == elem_step
    stride_bytes = elem_step * mybir.dt.size(in_ap.dtype)
    stride_bytes_256 = exact_div(stride_bytes, 256)
    _in_ap = eng.lower_ap_dma(in_ap, for_custom_bir_dma=True)
    _idxs_ap = eng.lower_ap(idxs_ap)
    _out_ap = eng.lower_ap(out_ap)
    return eng.add_instruction(
        mybir.InstDMAGatherAnt(
            name=nc.get_next_instruction_name(),
            ins=[*_in_ap, _idxs_ap, eng.lower_val_access(eng.to_reg(num_idxs))],
            outs=[_out_ap],
            transpose=False,
            num_idxs=num_idxs,
            elem_size=elem_size,
            stride_bytes_256=stride_bytes_256,
            gen_mode=0,
            single_packet=False,
            queue_num=0,
            sbuf_tokens_per_rank=0,
            sbuf_free_dim_per_rank=0,
            sbuf_free_dim_pad_per_rank=0,
            sbuf_byte_offset=0,
        )
    )


PITCH = 128  # fp16 elems per scratch row = 256B (ISA stride granularity)


def _build(Ks, SUMK):
    """Build + compile the (identical-across-cores) Bass program."""
    from concourse import bass, bacc, mybir, tile
    from concourse.masks import make_identity

    ck = (Ks, SUMK)
    if ck in _CACHE:
        return _CACHE[ck]

    f32 = mybir.dt.float32
    f16 = mybir.dt.float16
    i16 = mybir.dt.int16

    nc = bacc.Bacc(
        "TRN2",
        target_bir_lowering=False,
        debug=False,
        enable_asserts=False,
        num_devices=NCORES,
    )

    x32 = nc.dram_tensor("x32", [N, D], f32, kind="ExternalInput").ap()
    idx_d = nc.dram_tensor("idx", [P, 8 * SUMK], i16, kind="ExternalInput").ap()
    inv_d = nc.dram_tensor("inv", [P, NTILES], f32, kind="ExternalInput").ap()
    w_d = nc.dram_tensor("w", [D, D], f32, kind="ExternalInput").ap()
    bias_d = nc.dram_tensor("biasrep", [P, D], f32, kind="ExternalInput").ap()
    out_d = nc.dram_tensor("out", [RPC, D], f32, kind="ExternalOutput").ap()
    x16_d = nc.dram_tensor("x16s", [N + 1, PITCH], f16, kind="Internal").ap()

    offs = np.concatenate([[0], np.cumsum(Ks)]).astype(np.int64)

    with tile.TileContext(nc) as tc, ExitStack() as ctx:
        const = ctx.enter_context(tc.tile_pool(name="const", bufs=1))
        gp = ctx.enter_context(tc.tile_pool(name="gp", bufs=3))
        sp = ctx.enter_context(tc.tile_pool(name="sp", bufs=3))
        tp = ctx.enter_context(tc.tile_pool(name="tp", bufs=3))
        op_ = ctx.enter_context(tc.tile_pool(name="op", bufs=3))
        ppt = ctx.enter_context(tc.tile_pool(name="ppt", bufs=2, space="PSUM"))
        ppm = ctx.enter_context(tc.tile_pool(name="ppm", bufs=2, space="PSUM"))

        # constants
        w_sb = const.tile([D, D], f32)
        nc.sync.dma_start(out=w_sb[:], in_=w_d[:])
        bias_sb = const.tile([P, D], f32)
        nc.sync.dma_start(out=bias_sb[:], in_=bias_d[:])
        inv_sb = const.tile([P, NTILES], f32)
        nc.sync.dma_start(out=inv_sb[:], in_=inv_d[:])
        idx_sb = const.tile([P, 8 * SUMK], i16)
        nc.sync.dma_start(out=idx_sb[:], in_=idx_d[:])
        ident = const.tile([P, P], f32)
        make_identity(nc, ident[:])

        # zero the pad row of the fp16 scratch
        zrow = const.tile([1, D], f16)
        nc.vector.memset(zrow[:], 0.0)
        nc.sync.dma_start(out=x16_d[ZROW:ZROW + 1, 0:D], in_=zrow[:])

        # cast+pad x f32 -> fp16 into 256B-pitch scratch rows (SWDGE cast,
        # strided dst; split to stay under the per-DMA descriptor cap)
        NSPLIT = 4
        H = N // NSPLIT
        for i in range(NSPLIT):
            nc.gpsimd.dma_start(
                out=x16_d[i * H:(i + 1) * H, 0:D], in_=x32[i * H:(i + 1) * H, :]
            )

        ngroups = NTILES // GROUP
        for g in range(ngroups):
            tlist = list(range(g * GROUP, (g + 1) * GROUP))
            og = int(offs[tlist[0]])
            Kg = int(sum(Ks[t] for t in tlist))
            G = gp.tile([P, Kg * D], f16, tag="G")
            _emit_dma_gather(
                nc,
                out_ap=G[:].rearrange("p (k d) -> p k d", d=D),
                in_ap=x16_d[:, 0:D],
                idxs_ap=idx_sb[:, 8 * og:8 * (og + Kg)],
                num_idxs=P * Kg,
                elem_size=D,
                elem_step=PITCH,
            )

            lo = 0
            for t in tlist:
                K = Ks[t]
                Gt = G[:, lo * D:(lo + K) * D]
                lo += K

                # halving-tree segment sum over the K slot blocks (fp16),
                # final level lands in f32
                S = sp.tile([P, D], f32, tag="S")
                cur = K
                while cur > 2:
                    if cur % 2 == 1:
                        nc.vector.tensor_add(
                            out=Gt[:, 0:D],
                            in0=Gt[:, 0:D],
                            in1=Gt[:, (cur - 1) * D:cur * D],
                        )
                        cur -= 1
                    else:
                        m = cur // 2
                        nc.vector.tensor_add(
                            out=Gt[:, 0:m * D],
                            in0=Gt[:, 0:m * D],
                            in1=Gt[:, m * D:2 * m * D],
                        )
                        cur = m
                nc.vector.tensor_add(out=S[:], in0=Gt[:, 0:D], in1=Gt[:, D:2 * D])

                # S^T via PE, then (S @ W) via PE
                pT = ppt.tile([D, P], f32, tag="pT")
                nc.tensor.transpose(out=pT[:], in_=S[:], identity=ident[:])
                ST = tp.tile([D, P], f32, tag="ST")
                nc.scalar.copy(out=ST[:], in_=pT[:])
                pO = ppm.tile([P, D], f32, tag="pO")
                nc.tensor.matmul(
                    out=pO[:], lhsT=ST[:], rhs=w_sb[:], start=True, stop=True
                )

                # scale by 1/(deg+1) (per-partition), + bias, store
                O = op_.tile([P, D], f32, tag="O")
                nc.scalar.activation(
                    out=O[:],
                    in_=pO[:],
                    func=mybir.ActivationFunctionType.Copy,
                    scale=inv_sb[:, t:t + 1],
                )
                nc.vector.tensor_add(out=O[:], in0=O[:], in1=bias_sb[:])
                nc.sync.dma_start(out=out_d[t * P:(t + 1) * P, :], in_=O[:])

    nc.compile()
    _CACHE[ck] = nc
    return nc


def kernel(**inputs):
    global LAST_RESULTS
    from concourse import bass_utils

    x = np.ascontiguousarray(np.asarray(inputs["x"], dtype=np.float32))
    edge_index = np.asarray(inputs["edge_index"])
    weight = np.ascontiguousarray(np.asarray(inputs["weight"], dtype=np.float32))
    bias = np.asarray(inputs["bias"], dtype=np.float32)

    prep = _preprocess(edge_index)
    nc = _build(prep["Ks"], prep["SUMK"])

    bias_rep = np.ascontiguousarray(np.broadcast_to(bias[None, :], (P, D)))
    in_maps = [
        {
            "x32": x,
            "idx": prep["idx"][c],
            "inv": prep["inv"][c],
            "w": weight,
            "biasrep": bias_rep,
        }
        for c in range(NCORES)
    ]

    res = bass_utils.run_bass_kernel_spmd(
        nc, in_maps, core_ids=list(range(NCORES)), trace=_TRACE
    )
    LAST_RESULTS = res

    out = np.empty((N, D), dtype=np.float32)
    for c in range(NCORES):
        out[prep["perm"][c]] = res.results[c]["out"]
    return out


# revision 8
# speedup vs baseline: 1.5977x; 1.5977x over previous
"""GCNConv on 8 TRN2 NeuronCores.

out = rownorm(A + I) @ (x @ W) + b   with A = dense scatter (set semantics)
    = [per dst row r: (sum_{c in dedup(nbr(r))} x[c] + x[r]) / (deg(r)+1)] @ W + b

Strategy (1D node partition, per the sharding hint):
  - host: dedup edges, partition dst rows into 8 contiguous blocks of 2048,
    degree-sort rows inside each core block into 16 tiles of 128 rows,
    build a padded-CSR gather-index array [128, sum(K_t)] per core
    (pad slots point at a zeroed row), plus 1/(deg+1) per row.
  - device (identical program on all 8 cores, different data):
      * cast x f32 -> fp16 into a DRAM scratch (one SWDGE cast DMA)
      * per 2-tile group: one indirect-DMA gather of neighbor rows into
        SBUF [128, K*32] fp16 (one descriptor per edge slot)
      * DVE halving-tree segment sum -> S [128,32] f32
      * PE transpose -> S^T, PE matmul S@W, scalar scale by 1/(deg+1),
        DVE bias add, DMA out
  - host: inverse-permute the 8x2048 row blocks into the full output.
"""

import numpy as np
from contextlib import ExitStack

N = 16384
E = 524288
D = 32
P = 128
NCORES = 8
RPC = N // NCORES          # rows per core = 2048
NTILES = RPC // P          # 16 tiles of 128 rows per core
GROUP = 2                  # tiles per gather instruction
ZROW = N                   # index of the zeroed pad row in the fp16 scratch

_CACHE = {}
_PREP_CACHE = {}
LAST_RESULTS = None        # BassKernelResults of the last run (for test.py)
_TRACE = False             # test.py can flip this for a profiled run


def _preprocess(edge_index):
    """Dedup edges, build per-core degree-sorted padded-CSR gather schedule."""
    ei = np.asarray(edge_index)
    key = ei.tobytes()
    if key in _PREP_CACHE:
        return _PREP_CACHE[key]

    dst = ei[0].astype(np.int64)
    src = ei[1].astype(np.int64)
    keys = np.unique(dst * N + src)          # set semantics
    d = (keys // N).astype(np.int64)
    s = (keys % N).astype(np.int32)
    rowptr = np.searchsorted(d, np.arange(N + 1)).astype(np.int64)
    deg = np.diff(rowptr)                    # distinct out-neighbors per row
    slots = (deg + 1).astype(np.int64)       # + self loop
    inv = (1.0 / slots).astype(np.float32)

    # per-core degree-descending row order
    perms = []
    for c in range(NCORES):
        rows = np.arange(c * RPC, (c + 1) * RPC)
        order = np.argsort(-slots[rows], kind="stable")
        perms.append(rows[order])

    # shared (SPMD) per-tile pad width: max slots across cores in that tile
    Ks = []
    for t in range(NTILES):
        m = max(int(slots[perms[c][t * P]]) for c in range(NCORES))
        Ks.append(max(m, 2))
    Ks = tuple(Ks)
    offs = np.concatenate([[0], np.cumsum(Ks)]).astype(np.int64)
    SUMK = int(offs[-1])

    ngroups = NTILES // GROUP
    idx_arrs, inv_arrs = [], []
    for c in range(NCORES):
        plain = np.full((P, SUMK), ZROW, np.int16)
        invt = np.zeros((P, NTILES), np.float32)
        pc = perms[c]
        for t in range(NTILES):
            o = int(offs[t])
            for p in range(P):
                r = int(pc[t * P + p])
                a, b = rowptr[r], rowptr[r + 1]
                k = int(b - a)
                plain[p, o:o + k] = s[a:b]
                plain[p, o + k] = r          # self loop slot
                invt[p, t] = inv[r]
        # dma_gather index format: per gather group, gathered position
        # i = j*128 + p reads wrapped[i%16, i//16]; wrapped block for group g
        # occupies columns [8*off_g, 8*(off_g+Kg)); replicated to all 128
        # partitions (one copy per GPSIMD core's partition group).
        idxw = np.empty((16, 8 * SUMK), np.int16)
        for g in range(ngroups):
            og = int(offs[g * GROUP])
            Kg = int(offs[(g + 1) * GROUP] - offs[g * GROUP])
            block = plain[:, og:og + Kg]          # [128, Kg]
            flat = block.T.reshape(-1)            # flat[j*128+p] = block[p, j]
            idxw[:, 8 * og:8 * (og + Kg)] = flat.reshape(-1, 16).T
        idx_arrs.append(np.ascontiguousarray(np.tile(idxw, (8, 1))))
        inv_arrs.append(invt)

    prep = {
        "Ks": Ks,
        "offs": offs,
        "SUMK": SUMK,
        "idx": idx_arrs,
        "inv": inv_arrs,
        "perm": perms,
    }
    _PREP_CACHE[key] = prep
    return prep


def _emit_dma_gather(nc, out_ap, in_ap, idxs_ap, num_idxs, elem_size, elem_step,
                     queue_num=0):
    """bass.dma_gather minus its elem_size_bytes%256 assert (that restriction
    is transpose-only; the real ISA constraint is the source stride, which is
    encoded in 256B units and satisfied by the 256B-pitch scratch)."""
    from concourse import mybir
    from concourse._compat import exact_div

    eng = nc.gpsimd
    assert in_ap.ap[0][0] == elem_step
    stride_bytes = elem_step * mybir.dt.size(in_ap.dtype)
    stride_bytes_256 = exact_div(stride_bytes, 256)
    _in_ap = eng.lower_ap_dma(in_ap, for_custom_bir_dma=True)
    _idxs_ap = eng.lower_ap(idxs_ap)
    _out_ap = eng.lower_ap(out_ap)
    return eng.add_instruction(
        mybir.InstDMAGatherAnt(
            name=nc.get_next_instruction_name(),
            ins=[*_in_ap, _idxs_ap, eng.lower_val_access(eng.to_reg(num_idxs))],
            outs=[_out_ap],
            transpose=False,
            num_idxs=num_idxs,
            elem_size=elem_size,
            stride_bytes_256=stride_bytes_256,
            gen_mode=0,
            single_packet=False,
            queue_num=queue_num,
            sbuf_tokens_per_rank=0,
            sbuf_free_dim_per_rank=0,
            sbuf_free_dim_pad_per_rank=0,
            sbuf_byte_offset=0,
        )
    )


PITCH = 128  # fp16 elems per scratch row = 256B (ISA stride granularity)


def _build(Ks, SUMK):
    """Build + compile the (identical-across-cores) Bass program."""
    from concourse import bass, bacc, mybir, tile
    from concourse.masks import make_identity

    ck = (Ks, SUMK)
    if ck in _CACHE:
        return _CACHE[ck]

    f32 = mybir.dt.float32
    f16 = mybir.dt.float16
    i16 = mybir.dt.int16

    nc = bacc.Bacc(
        "TRN2",
        target_bir_lowering=False,
        debug=False,
        enable_asserts=False,
        num_devices=NCORES,
        num_swdge_queues=4,
    )

    x32 = nc.dram_tensor("x32", [N, D], f32, kind="ExternalInput").ap()
    idx_d = nc.dram_tensor("idx", [P, 8 * SUMK], i16, kind="ExternalInput").ap()
    inv_d = nc.dram_tensor("inv", [P, NTILES], f32, kind="ExternalInput").ap()
    w_d = nc.dram_tensor("w", [D, D], f32, kind="ExternalInput").ap()
    bias_d = nc.dram_tensor("biasrep", [P, D], f32, kind="ExternalInput").ap()
    out_d = nc.dram_tensor("out", [RPC, D], f32, kind="ExternalOutput").ap()
    x16_d = nc.dram_tensor("x16s", [N + 1, PITCH], f16, kind="Internal").ap()

    offs = np.concatenate([[0], np.cumsum(Ks)]).astype(np.int64)

    with tile.TileContext(nc) as tc, ExitStack() as ctx:
        const = ctx.enter_context(tc.tile_pool(name="const", bufs=1))
        gp = ctx.enter_context(tc.tile_pool(name="gp", bufs=3))
        sp = ctx.enter_context(tc.tile_pool(name="sp", bufs=3))
        tp = ctx.enter_context(tc.tile_pool(name="tp", bufs=3))
        op_ = ctx.enter_context(tc.tile_pool(name="op", bufs=3))
        ppt = ctx.enter_context(tc.tile_pool(name="ppt", bufs=2, space="PSUM"))
        ppm = ctx.enter_context(tc.tile_pool(name="ppm", bufs=2, space="PSUM"))

        # constants
        w_sb = const.tile([D, D], f32)
        nc.sync.dma_start(out=w_sb[:], in_=w_d[:])
        bias_sb = const.tile([P, D], f32)
        nc.sync.dma_start(out=bias_sb[:], in_=bias_d[:])
        inv_sb = const.tile([P, NTILES], f32)
        nc.sync.dma_start(out=inv_sb[:], in_=inv_d[:])
        idx_sb = const.tile([P, 8 * SUMK], i16)
        nc.sync.dma_start(out=idx_sb[:], in_=idx_d[:])
        ident = const.tile([P, P], f32)
        make_identity(nc, ident[:])

        # zero the pad row of the fp16 scratch
        zrow = const.tile([1, D], f16)
        nc.vector.memset(zrow[:], 0.0)
        nc.sync.dma_start(out=x16_d[ZROW:ZROW + 1, 0:D], in_=zrow[:])

        # cast+pad x f32 -> fp16 into 256B-pitch scratch rows (SWDGE cast,
        # strided dst; split to stay under the per-DMA descriptor cap)
        NSPLIT = 4
        H = N // NSPLIT
        for i in range(NSPLIT):
            nc.gpsimd.dma_start(
                out=x16_d[i * H:(i + 1) * H, 0:D], in_=x32[i * H:(i + 1) * H, :]
            )

        ngroups = NTILES // GROUP
        for g in range(ngroups):
            tlist = list(range(g * GROUP, (g + 1) * GROUP))
            og = int(offs[tlist[0]])
            Kg = int(sum(Ks[t] for t in tlist))
            G = gp.tile([P, Kg * D], f16, tag="G")
            _emit_dma_gather(
                nc,
                out_ap=G[:].rearrange("p (k d) -> p k d", d=D),
                in_ap=x16_d[:, 0:D],
                idxs_ap=idx_sb[:, 8 * og:8 * (og + Kg)],
                num_idxs=P * Kg,
                elem_size=D,
                elem_step=PITCH,
                queue_num=g % 4,
            )

            lo = 0
            for t in tlist:
                K = Ks[t]
                Gt = G[:, lo * D:(lo + K) * D]
                lo += K

                # halving-tree segment sum over the K slot blocks (fp16),
                # final level lands in f32
                S = sp.tile([P, D], f32, tag="S")
                cur = K
                while cur > 2:
                    if cur % 2 == 1:
                        nc.vector.tensor_add(
                            out=Gt[:, 0:D],
                            in0=Gt[:, 0:D],
                            in1=Gt[:, (cur - 1) * D:cur * D],
                        )
                        cur -= 1
                    else:
                        m = cur // 2
                        nc.vector.tensor_add(
                            out=Gt[:, 0:m * D],
                            in0=Gt[:, 0:m * D],
                            in1=Gt[:, m * D:2 * m * D],
                        )
                        cur = m
                nc.vector.tensor_add(out=S[:], in0=Gt[:, 0:D], in1=Gt[:, D:2 * D])

                # S^T via PE, then (S @ W) via PE
                pT = ppt.tile([D, P], f32, tag="pT")
                nc.tensor.transpose(out=pT[:], in_=S[:], identity=ident[:])
                ST = tp.tile([D, P], f32, tag="ST")
                nc.scalar.copy(out=ST[:], in_=pT[:])
                pO = ppm.tile([P, D], f32, tag="pO")
                nc.tensor.matmul(
                    out=pO[:], lhsT=ST[:], rhs=w_sb[:], start=True, stop=True
                )

                # scale by 1/(deg+1) (per-partition), + bias, store
                O = op_.tile([P, D], f32, tag="O")
                nc.scalar.activation(
                    out=O[:],
                    in_=pO[:],
                    func=mybir.ActivationFunctionType.Copy,
                    scale=inv_sb[:, t:t + 1],
                )
                nc.vector.tensor_add(out=O[:], in0=O[:], in1=bias_sb[:])
                nc.sync.dma_start(out=out_d[t * P:(t + 1) * P, :], in_=O[:])

    nc.compile()
    _CACHE[ck] = nc
    return nc


def kernel(**inputs):
    global LAST_RESULTS
    from concourse import bass_utils

    x = np.ascontiguousarray(np.asarray(inputs["x"], dtype=np.float32))
    edge_index = np.asarray(inputs["edge_index"])
    weight = np.ascontiguousarray(np.asarray(inputs["weight"], dtype=np.float32))
    bias = np.asarray(inputs["bias"], dtype=np.float32)

    prep = _preprocess(edge_index)
    nc = _build(prep["Ks"], prep["SUMK"])

    bias_rep = np.ascontiguousarray(np.broadcast_to(bias[None, :], (P, D)))
    in_maps = [
        {
            "x32": x,
            "idx": prep["idx"][c],
            "inv": prep["inv"][c],
            "w": weight,
            "biasrep": bias_rep,
        }
        for c in range(NCORES)
    ]

    res = bass_utils.run_bass_kernel_spmd(
        nc, in_maps, core_ids=list(range(NCORES)), trace=_TRACE
    )
    LAST_RESULTS = res

    out = np.empty((N, D), dtype=np.float32)
    for c in range(NCORES):
        out[prep["perm"][c]] = res.results[c]["out"]
    return out
